# revision 3
# baseline (speedup 1.0000x reference)
"""Causal single-head attention (N=4096, D=1024) on 8 TRN2 NeuronCores.

Sharding: query rows are striped across cores (core i owns global rows
{8*m + i}), which makes the causal workload — and therefore the SPMD
instruction stream — identical on every core.  K/V projections are computed
for each core's own 512-row stripe and AllGathered; Q projection overlaps
the gather.  Scores are computed transposed (S^T = K @ Q^T) so the softmax
normalizer is a ones-column matmul on the PE and P^T is directly the
stationary operand of the A@V matmuls: no on-chip transposes anywhere.

softmax(s) is computed as exp(s/32 - 8) / sum(exp(s/32 - 8)): the constant
shift cancels exactly in the normalization, and |s/32| stays far below the
fp32 exp range for these inputs, so this matches the reference's
max-subtracted softmax to fp32 accuracy.  Masked (j > r) entries are zeroed
exactly via host-supplied 0/1 masks, matching the reference's -10000 fill
(exp(-10000 + ...) underflows to exactly 0 in fp32).
"""

import numpy as np
import ml_dtypes

import concourse.bacc as bacc
import concourse.mybir as mybir
import concourse.tile as tile
from concourse.bass_utils import run_bass_kernel_spmd

N = 4096
D = 1024
NC = 8
RPC = N // NC          # 512 query rows per core
NT = RPC // 128        # 4 row-tiles of 128 per core
SCALE = 1.0 / 32.0     # 1/sqrt(D)
SHIFT = -8.0           # constant softmax shift (cancels in normalization)

BF16 = mybir.dt.bfloat16
F32 = mybir.dt.float32


def build_nc():
    nc = bacc.Bacc("TRN2", target_bir_lowering=False, num_devices=NC)
    Exp = mybir.ActivationFunctionType.Exp

    # Per-core inputs.  *T tensors are host-pre-transposed so every matmul
    # operand is already in its PE layout.
    qxT = nc.dram_tensor("qxT", [D, RPC], BF16, kind="ExternalInput")
    kxT = nc.dram_tensor("kxT", [D, RPC], BF16, kind="ExternalInput")
    vxT = nc.dram_tensor("vxT", [D, RPC], BF16, kind="ExternalInput")
    wqT = nc.dram_tensor("wqT", [D, D], BF16, kind="ExternalInput")
    wkT = nc.dram_tensor("wkT", [D, D], BF16, kind="ExternalInput")
    wvT = nc.dram_tensor("wvT", [D, D], BF16, kind="ExternalInput")
    # mask[jp, c, rl] = 1.0 where key row 8*jp+c <= query row 8*rl+i
    maskin = nc.dram_tensor("maskin", [128, 8, 128], BF16, kind="ExternalInput")
    y = nc.dram_tensor("y", [RPC, D], F32, kind="ExternalOutput")

    # Collective bounce buffers (collectives can't touch I/O tensors).
    k_loc = nc.dram_tensor("k_loc", [8, 128, RPC], BF16)            # (do, p, m)
    v_loc = nc.dram_tensor("v_loc", [NT, 128, D], BF16)             # (mt, p, d)
    k_all = nc.dram_tensor("k_all", [NC, 8, 128, RPC], BF16)
    v_all = nc.dram_tensor("v_all", [NC, NT, 128, D], BF16)

    rg = [list(range(NC))]

    with tile.TileContext(nc) as tc:
        with (
            tc.tile_pool(name="const", bufs=1) as const,
            tc.tile_pool(name="proj", bufs=3) as proj,
            tc.tile_pool(name="sb", bufs=3) as sb,
            tc.tile_pool(name="yp", bufs=2) as yp,
            tc.tile_pool(name="ps", bufs=2, space="PSUM") as ps,
            tc.tile_pool(name="acc", bufs=1, space="PSUM") as accp,
        ):
            # ---- phase A: K projection, then AllGather K ----
            wk_sb = const.tile([128, 8, D], BF16, tag="wk")
            nc.sync.dma_start(wk_sb[:], wkT.rearrange("(ct p) o -> p ct o", p=128))
            kx_sb = const.tile([128, 8, RPC], BF16, tag="kx")
            nc.sync.dma_start(kx_sb[:], kxT.rearrange("(ct p) m -> p ct m", p=128))

            # kT_loc[do, m] = sum_c WkT[c, do] * kxT[c, m]
            for do in range(8):
                pk = ps.tile([128, 512], F32, tag="mm")
                for ct in range(8):
                    nc.tensor.matmul(
                        pk[:], wk_sb[:, ct, 128 * do:128 * (do + 1)], kx_sb[:, ct, :],
                        start=(ct == 0), stop=(ct == 7))
                ko = proj.tile([128, 512], BF16, tag="ko")
                nc.vector.tensor_copy(ko[:], pk[:])
                nc.sync.dma_start(k_loc[do], ko[:])

            nc.gpsimd.collective_compute(
                "AllGather", mybir.AluOpType.bypass, replica_groups=rg,
                ins=[k_loc[:].opt()], outs=[k_all[:].opt()])

            # ---- phase B: V projection, AllGather V; Q projection overlaps ----
            wv_sb = const.tile([128, 8, D], BF16, tag="wv")
            nc.sync.dma_start(wv_sb[:], wvT.rearrange("(ct p) o -> p ct o", p=128))
            vx_sb = const.tile([128, 8, RPC], BF16, tag="vx")
            nc.sync.dma_start(vx_sb[:], vxT.rearrange("(ct p) m -> p ct m", p=128))

            # v_loc[m, d] = sum_c vxT[c, m] * WvT[c, d]
            for mt in range(NT):
                vo = proj.tile([128, D], BF16, tag="vo")
                for h in range(2):
                    pv = ps.tile([128, 512], F32, tag="mm")
                    for ct in range(8):
                        nc.tensor.matmul(
                            pv[:], vx_sb[:, ct, 128 * mt:128 * (mt + 1)],
                            wv_sb[:, ct, 512 * h:512 * (h + 1)],
                            start=(ct == 0), stop=(ct == 7))
                    nc.vector.tensor_copy(vo[:, 512 * h:512 * (h + 1)], pv[:])
                nc.sync.dma_start(v_loc[mt], vo[:])

            nc.gpsimd.collective_compute(
                "AllGather", mybir.AluOpType.bypass, replica_groups=rg,
                ins=[v_loc[:].opt()], outs=[v_all[:].opt()])

            wq_sb = const.tile([128, 8, D], BF16, tag="wq")
            nc.sync.dma_start(wq_sb[:], wqT.rearrange("(ct p) o -> p ct o", p=128))
            qx_sb = const.tile([128, 8, RPC], BF16, tag="qx")
            nc.sync.dma_start(qx_sb[:], qxT.rearrange("(ct p) m -> p ct m", p=128))

            # qT_sb[p, do, r] = q^T[(128*do+p), r], kept resident in SBUF
            qT_sb = const.tile([128, 8, RPC], BF16, tag="qt")
            for do in range(8):
                pq = ps.tile([128, 512], F32, tag="mm")
                for ct in range(8):
                    nc.tensor.matmul(
                        pq[:], wq_sb[:, ct, 128 * do:128 * (do + 1)], qx_sb[:, ct, :],
                        start=(ct == 0), stop=(ct == 7))
                nc.vector.tensor_copy(qT_sb[:, do, :], pq[:])

            mask_sb = const.tile([128, 8, 128], BF16, tag="mask")
            nc.sync.dma_start(mask_sb[:], maskin[:])
            ones_sb = const.tile([128, 1], BF16, tag="ones")
            nc.vector.memset(ones_sb[:], 1.0)
            shift_sb = const.tile([128, 1], F32, tag="shift")
            nc.vector.memset(shift_sb[:], SHIFT)

            # ---- phase C: attention, row-tile pairs (2,3) then (0,1) ----
            for t0 in (2, 0):
                t1 = t0 + 1
                # (u, c, kind): kind 0 = fully-causal block (both row tiles),
                # kind 1 = diagonal band of t0 (both), kind 2 = diagonal of t1
                jts = [(u, c, 0) for u in range(t0) for c in range(8)]
                jts += [(t0, c, 1) for c in range(8)]
                jts += [(t1, c, 2) for c in range(8)]
                last_a = 8 * t0 + 7
                last_b = len(jts) - 1

                acc_a = accp.tile([128, D], F32, tag="acc_a")
                acc_b = accp.tile([128, D], F32, tag="acc_b")
                den_a = accp.tile([128, 1], F32, tag="den_a")
                den_b = accp.tile([128, 1], F32, tag="den_b")

                for idx, (u, c, kind) in enumerate(jts):
                    kt = sb.tile([128, 8, 128], BF16, tag="kt")
                    nc.sync.dma_start(
                        kt[:],
                        k_all[c].rearrange("dd p m -> p dd m")[:, :, 128 * u:128 * (u + 1)])
                    vt = sb.tile([128, D], BF16, tag="vt")
                    nc.gpsimd.dma_start(vt[:], v_all[c, u])

                    w = 256 if kind < 2 else 128
                    rc0 = 128 * t0 if kind < 2 else 128 * t1
                    st = ps.tile([128, 256], F32, tag="mm")
                    for dd in range(8):
                        nc.tensor.matmul(
                            st[:, :w], kt[:, dd, :], qT_sb[:, dd, rc0:rc0 + w],
                            start=(dd == 0), stop=(dd == 7))

                    p = sb.tile([128, 256], BF16, tag="p")
                    nc.scalar.activation(p[:, :w], st[:, :w], Exp,
                                         bias=shift_sb[:], scale=SCALE)
                    if kind >= 1:
                        nc.vector.tensor_mul(p[:, 0:128], p[:, 0:128], mask_sb[:, c, :])

                    subs = ((acc_a, den_a, 0, idx == 0, idx == last_a),
                            (acc_b, den_b, 1, idx == 0, idx == last_b)) if w == 256 else \
                           ((acc_b, den_b, 0, idx == 0, idx == last_b),)
                    for acc, den, si, first, last in subs:
                        pt = p[:, 128 * si:128 * (si + 1)]
                        nc.tensor.matmul(acc[:, 0:512], pt, vt[:, 0:512],
                                         start=first, stop=last)
                        nc.tensor.matmul(acc[:, 512:1024], pt, vt[:, 512:1024],
                                         start=first, stop=last)
                        nc.tensor.matmul(den[:], pt, ones_sb[:],
                                         start=first, stop=last)

                rec = sb.tile([128, 2], F32, tag="rec")
                nc.vector.reciprocal(rec[:, 0:1], den_a[:])
                nc.vector.reciprocal(rec[:, 1:2], den_b[:])
                for t, acc, col in ((t0, acc_a, 0), (t1, acc_b, 1)):
                    yo = yp.tile([128, D], F32, tag="yo")
                    nc.vector.tensor_scalar_mul(yo[:], acc[:], rec[:, col:col + 1])
                    nc.sync.dma_start(y[128 * t:128 * (t + 1), :], yo[:])

    nc.compile()
    return nc


_NC_CACHE = None


def _get_nc():
    global _NC_CACHE
    if _NC_CACHE is None:
        _NC_CACHE = build_nc()
    return _NC_CACHE


def make_in_maps(qx, kx, vx, Wq, Wk, Wv):
    bf = ml_dtypes.bfloat16
    wqT = np.ascontiguousarray(Wq.astype(np.float32).T.astype(bf))
    wkT = np.ascontiguousarray(Wk.astype(np.float32).T.astype(bf))
    wvT = np.ascontiguousarray(Wv.astype(np.float32).T.astype(bf))
    in_maps = []
    for i in range(NC):
        rows = np.arange(RPC) * NC + i
        jp = np.arange(128)[:, None, None]
        cc = np.arange(8)[None, :, None]
        rl = np.arange(128)[None, None, :]
        mask = (8 * jp + cc <= 8 * rl + i).astype(bf)
        in_maps.append({
            "qxT": np.ascontiguousarray(qx[rows].T.astype(bf)),
            "kxT": np.ascontiguousarray(kx[rows].T.astype(bf)),
            "vxT": np.ascontiguousarray(vx[rows].T.astype(bf)),
            "wqT": wqT, "wkT": wkT, "wvT": wvT,
            "maskin": np.ascontiguousarray(mask),
        })
    return in_maps


def assemble(results):
    out = np.empty((N, D), np.float32)
    for i in range(NC):
        out[np.arange(RPC) * NC + i] = results[i]["y"]
    return out


def kernel(qx, kx, vx, Wq, Wk, Wv):
    nc = _get_nc()
    in_maps = make_in_maps(qx, kx, vx, Wq, Wk, Wv)
    res = run_bass_kernel_spmd(nc, in_maps, core_ids=list(range(NC)))
    return assemble(res.results)


# revision 13
# speedup vs baseline: 860.7247x; 860.7247x over previous
"""Causal single-head attention (N=4096, D=1024) on 8 TRN2 NeuronCores.

Sharding: query rows are striped across cores (core i owns global rows
{8*m + i}), which makes the causal workload — and therefore the SPMD
instruction stream — identical on every core.  K/V projections are computed
for each core's own 512-row stripe and AllGathered in two chunks (key/value
row-blocks u{0,1} then u{2,3}, packed k+v per chunk) so attention on the
(0,1) row-tile pair starts after the first chunk while the second gathers;
Q projection overlaps the first gather.  Scores are computed transposed
(S^T = K @ Q^T) so the softmax normalizer is a ones-column matmul on the PE
and P^T is directly the stationary operand of the A@V matmuls: no on-chip
transposes anywhere.  Gathered K/V tiles are loaded in batched per-u-block
DMAs (1 descriptor-gen per 2MB instead of per 256KB) and the u{0,1} blocks
stay cached in SBUF across the two row-tile-pair passes.

softmax(s) is computed as exp(s/32 - 8) / sum(exp(s/32 - 8)): the constant
shift cancels exactly in the normalization, and |s/32| stays far below the
fp32 exp range for these inputs, so this matches the reference's
max-subtracted softmax to fp32 accuracy.  Masked (j > r) entries are zeroed
exactly via host-supplied 0/1 masks, matching the reference's -10000 fill
(exp(-10000 + ...) underflows to exactly 0 in fp32).
"""

import numpy as np
import ml_dtypes

import concourse.bacc as bacc
import concourse.mybir as mybir
import concourse.tile as tile
from concourse.bass_utils import run_bass_kernel_spmd

N = 4096
D = 1024
NC = 8
RPC = N // NC          # 512 query rows per core
NT = RPC // 128        # 4 row-tiles of 128 per core
SCALE = 1.0 / 32.0     # 1/sqrt(D)
SHIFT = -8.0           # constant softmax shift (cancels in normalization)

BF16 = mybir.dt.bfloat16
F32 = mybir.dt.float32


def build_nc(reps=1, rep_phases="all"):
    """reps>1 unrolls phases for slope-based device timing.
    rep_phases: "all" | "proj" | "ag" | "attn" | "dma" — which part repeats."""
    nc = bacc.Bacc("TRN2", target_bir_lowering=False, num_devices=NC)
    Exp = mybir.ActivationFunctionType.Exp

    # Per-core inputs.  *T tensors are host-pre-transposed so every matmul
    # operand is already in its PE layout.
    qxT = nc.dram_tensor("qxT", [D, RPC], BF16, kind="ExternalInput")
    kxT = nc.dram_tensor("kxT", [D, RPC], BF16, kind="ExternalInput")
    vxT = nc.dram_tensor("vxT", [D, RPC], BF16, kind="ExternalInput")
    wqT = nc.dram_tensor("wqT", [D, D], BF16, kind="ExternalInput")
    wkT = nc.dram_tensor("wkT", [D, D], BF16, kind="ExternalInput")
    wvT = nc.dram_tensor("wvT", [D, D], BF16, kind="ExternalInput")
    # mask[jp, c, rl] = 1.0 where key row 8*jp+c <= query row 8*rl+i
    maskin = nc.dram_tensor("maskin", [128, 8, 128], BF16, kind="ExternalInput")
    y = nc.dram_tensor("y", [RPC, D], F32, kind="ExternalOutput")

    # Collective bounce buffers (collectives can't touch I/O tensors).
    # kv_loc[u, 0] = k^T block u as (p, do*128+m); kv_loc[u, 1] = v block u.
    # Gathered in two chunks: a = u in {0,1}, b = u in {2,3} (rank-major).
    kv_loc = nc.dram_tensor("kv_loc", [NT, 2, 128, D], BF16)
    kv_all_a = nc.dram_tensor("kv_all_a", [NC, 2, 2, 128, D], BF16)
    kv_all_b = nc.dram_tensor("kv_all_b", [NC, 2, 2, 128, D], BF16)

    rg = [list(range(NC))]

    with tile.TileContext(nc) as tc:
        with (
            tc.tile_pool(name="const", bufs=1) as const,
            tc.tile_pool(name="wrot", bufs=2) as wrot_p,
            tc.tile_pool(name="xrot", bufs=2) as xrot_p,
            tc.tile_pool(name="proj", bufs=3) as proj,
            tc.tile_pool(name="sb", bufs=3) as sb,
            tc.tile_pool(name="kv", bufs=2) as kv,
            tc.tile_pool(name="vtc", bufs=1) as vtc,
            tc.tile_pool(name="vkv", bufs=2) as vkv,
            tc.tile_pool(name="pp", bufs=4) as pp,
            tc.tile_pool(name="yp", bufs=2) as yp,
            tc.tile_pool(name="ps", bufs=2, space="PSUM") as ps,
            tc.tile_pool(name="acc", bufs=1, space="PSUM") as accp,
        ):
            ctx_pools = {"wrot": wrot_p, "xrot": xrot_p}
            wrot = ctx_pools["wrot"]
            xrot = ctx_pools["xrot"]

            def load_xT(dram, tag):
                t = xrot.tile([128, 8, RPC], BF16, tag="x")
                nc.sync.dma_start(t[:], dram.rearrange("(ct p) m -> p ct m", p=128))
                return t

            def load_w(dram, tag):
                t = wrot.tile([128, 8, D], BF16, tag="w")
                nc.sync.dma_start(t[:], dram.rearrange("(ct p) o -> p ct o", p=128))
                return t

            def emit_kproj_half(wk_sb, kx_sb, h):
                # k^T for m in [256h, 256h+256) = u blocks {2h, 2h+1}
                for do in range(8):
                    pk = ps.tile([128, 256], F32, tag="mm")
                    for ct in range(8):
                        nc.tensor.matmul(
                            pk[:], wk_sb[:, ct, 128 * do:128 * (do + 1)],
                            kx_sb[:, ct, 256 * h:256 * (h + 1)],
                            start=(ct == 0), stop=(ct == 7))
                    ko = proj.tile([128, 256], BF16, tag="ko")
                    nc.vector.tensor_copy(ko[:], pk[:])
                    nc.sync.dma_start(
                        kv_loc[2 * h:2 * h + 2, 0, :, 128 * do:128 * (do + 1)]
                        .rearrange("u p m -> p u m"),
                        ko[:].rearrange("p (u m) -> p u m", u=2))

            def emit_qproj():
                wq_sb = load_w(wqT, "wq")
                qx_sb = load_xT(qxT, "qx")
                # qT_sb[p, do, r] = q^T[(128*do+p), r], kept resident in SBUF.
                # r-half 0 (rows of tiles 0/1) first so pair (0,1) QK can start.
                qT_sb = const.tile([128, 8, RPC], BF16, tag="qt")
                for h in range(2):
                    for do in range(8):
                        pq = ps.tile([128, 256], F32, tag="mm")
                        for ct in range(8):
                            nc.tensor.matmul(
                                pq[:], wq_sb[:, ct, 128 * do:128 * (do + 1)],
                                qx_sb[:, ct, 256 * h:256 * (h + 1)],
                                start=(ct == 0), stop=(ct == 7))
                        nc.vector.tensor_copy(
                            qT_sb[:, do, 256 * h:256 * (h + 1)], pq[:])
                return qT_sb

            def load_vw(tag):
                return load_w(wvT, tag), load_xT(vxT, tag + "x")

            def emit_vproj_u(wv_sb, vx_sb, mt):
                # v_loc[m, d] = sum_c vxT[c, m] * WvT[c, d], one 128-row block
                vo = proj.tile([128, D], BF16, tag="vo")
                for h in range(2):
                    pv = ps.tile([128, 512], F32, tag="mm")
                    for ct in range(8):
                        nc.tensor.matmul(
                            pv[:], vx_sb[:, ct, 128 * mt:128 * (mt + 1)],
                            wv_sb[:, ct, 512 * h:512 * (h + 1)],
                            start=(ct == 0), stop=(ct == 7))
                    nc.vector.tensor_copy(vo[:, 512 * h:512 * (h + 1)], pv[:])
                nc.sync.dma_start(kv_loc[mt, 1], vo[:])

            def emit_ag_chunk(half):
                outb = kv_all_a if half == 0 else kv_all_b
                nc.gpsimd.collective_compute(
                    "AllGather", mybir.AluOpType.bypass, replica_groups=rg,
                    ins=[kv_loc[2 * half:2 * half + 2].opt()], outs=[outb[:].opt()])

            def emit_consts():
                mask_sb = const.tile([128, 8, 128], BF16, tag="mask")
                nc.sync.dma_start(mask_sb[:], maskin[:])
                ones_sb = const.tile([128, 1], BF16, tag="ones")
                nc.vector.memset(ones_sb[:], 1.0)
                shift_sb = const.tile([128, 1], F32, tag="shift")
                nc.vector.memset(shift_sb[:], SHIFT)
                return mask_sb, ones_sb, shift_sb

            def kv_src(u):
                buf = kv_all_a if u < 2 else kv_all_b
                return buf, u % 2

            def load_kt_u(u, cache):
                # one batched DMA for all 8 ranks' k^T block u: [p, c, dd, m]
                key = ("k", u)
                if key in cache:
                    return cache[key]
                buf, uu = kv_src(u)
                if u < 2:
                    kt = vtc.tile([128, NC, 8, 128], BF16, tag=f"ktu{u}")
                else:
                    kt = vkv.tile([128, NC, 8, 128], BF16, tag="ktu23")
                nc.sync.dma_start(
                    kt[:],
                    buf[:, uu, 0].rearrange("c p (dd m) -> p c dd m", dd=8))
                cache[key] = kt
                return kt

            def load_vt_u(u, cache):
                # one batched DMA for all 8 ranks' v block u: [p, c, d]
                if u in cache:
                    return cache[u]
                buf, uu = kv_src(u)
                if u < 2:
                    vt = vtc.tile([128, NC, D], BF16, tag=f"vtu{u}")
                else:
                    vt = vkv.tile([128, NC, D], BF16, tag="vtu23")
                nc.gpsimd.dma_start(
                    vt[:], buf[:, uu, 1].rearrange("c p d -> p c d"))
                cache[u] = vt
                return vt

            def emit_attn(qT_sb, mask_sb, ones_sb, shift_sb):
                vt_cache = {}
                # pair (0,1) first: it only needs the u{0,1} AG chunk
                for t0 in (0, 2):
                    t1 = t0 + 1
                    # (u, c, kind): kind 0 = full block (both row tiles),
                    # kind 1 = diagonal of t0 (both), kind 2 = diagonal of t1
                    jts = [(u, c, 0) for u in range(t0) for c in range(8)]
                    jts += [(t0, c, 1) for c in range(8)]
                    jts += [(t1, c, 2) for c in range(8)]
                    last_a = 8 * t0 + 7
                    last_b = len(jts) - 1

                    acc_a = accp.tile([128, D], F32, tag="acc_a")
                    acc_b = accp.tile([128, D], F32, tag="acc_b")
                    den_a = accp.tile([128, 1], F32, tag="den_a")
                    den_b = accp.tile([128, 1], F32, tag="den_b")

                    for idx, (u, c, kind) in enumerate(jts):
                        ktu = load_kt_u(u, vt_cache)
                        vtu = load_vt_u(u, vt_cache)
                        kt = ktu[:, c]
                        vt = vtu[:, c, :]

                        w = 256 if kind < 2 else 128
                        rc0 = 128 * t0 if kind < 2 else 128 * t1
                        st = ps.tile([128, 256], F32, tag="mm")
                        for dd in range(8):
                            nc.tensor.matmul(
                                st[:, :w], kt[:, dd, :], qT_sb[:, dd, rc0:rc0 + w],
                                start=(dd == 0), stop=(dd == 7))

                        p = pp.tile([128, 256], BF16, tag="p")
                        nc.scalar.activation(p[:, :w], st[:, :w], Exp,
                                             bias=shift_sb[:], scale=SCALE)
                        if kind >= 1:
                            nc.vector.tensor_mul(p[:, 0:128], p[:, 0:128],
                                                 mask_sb[:, c, :])

                        subs = ((acc_a, den_a, 0, idx == 0, idx == last_a),
                                (acc_b, den_b, 1, idx == 0, idx == last_b)) \
                            if w == 256 else \
                               ((acc_b, den_b, 0, idx == 0, idx == last_b),)
                        for acc, den, si, first, last in subs:
                            pt = p[:, 128 * si:128 * (si + 1)]
                            nc.tensor.matmul(acc[:, 0:512], pt, vt[:, 0:512],
                                             start=first, stop=last)
                            nc.tensor.matmul(acc[:, 512:1024], pt, vt[:, 512:1024],
                                             start=first, stop=last)
                            nc.tensor.matmul(den[:], pt, ones_sb[:],
                                             start=first, stop=last)

                    rec = sb.tile([128, 2], F32, tag="rec")
                    nc.vector.reciprocal(rec[:, 0:1], den_a[:])
                    nc.vector.reciprocal(rec[:, 1:2], den_b[:])
                    for t, acc, col in ((t0, acc_a, 0), (t1, acc_b, 1)):
                        yo = yp.tile([128, D], F32, tag="yo")
                        nc.vector.tensor_scalar_mul(yo[:], acc[:], rec[:, col:col + 1])
                        nc.sync.dma_start(y[128 * t:128 * (t + 1), :], yo[:])

            def emit_attn_dma_only():
                # same kt/vt DMA footprint as emit_attn, no compute
                cache = {}
                for t0 in (0, 2):
                    for u in range(t0 + 2):
                        load_vt_u(u, cache)
                        load_kt_u(u, cache)

            def emit_proj_and_ags():
                wk_sb = load_w(wkT, "wk")
                kx_sb = load_xT(kxT, "kx")
                wv_sb, vx_sb = load_vw("wv")
                for half in range(2):
                    emit_kproj_half(wk_sb, kx_sb, half)
                    emit_vproj_u(wv_sb, vx_sb, 2 * half)
                    emit_vproj_u(wv_sb, vx_sb, 2 * half + 1)
                    emit_ag_chunk(half)
                return emit_qproj()

            if rep_phases == "all":
                for _ in range(reps):
                    qT_sb = emit_proj_and_ags()
                    consts = emit_consts()
                    emit_attn(qT_sb, *consts)
            elif rep_phases == "proj":
                for _ in range(reps):
                    wk_sb = load_w(wkT, "wk")
                    kx_sb = load_xT(kxT, "kx")
                    wv_sb, vx_sb = load_vw("wv")
                    for half in range(2):
                        emit_kproj_half(wk_sb, kx_sb, half)
                    for u in range(NT):
                        emit_vproj_u(wv_sb, vx_sb, u)
                    qT_sb = emit_qproj()
                for half in range(2):
                    emit_ag_chunk(half)
                consts = emit_consts()
                emit_attn(qT_sb, *consts)
            elif rep_phases == "ag":
                qT_sb = emit_proj_and_ags()
                for _ in range(reps - 1):
                    for half in range(2):
                        emit_ag_chunk(half)
                consts = emit_consts()
                emit_attn(qT_sb, *consts)
            elif rep_phases == "attn":
                qT_sb = emit_proj_and_ags()
                consts = emit_consts()
                for _ in range(reps):
                    emit_attn(qT_sb, *consts)
            elif rep_phases == "dma":
                qT_sb = emit_proj_and_ags()
                consts = emit_consts()
                for _ in range(reps):
                    emit_attn_dma_only()
                emit_attn(qT_sb, *consts)
            else:
                raise ValueError(rep_phases)

    nc.compile()
    return nc


_NC_CACHE = None


def _get_nc():
    global _NC_CACHE
    if _NC_CACHE is None:
        _NC_CACHE = build_nc()
    return _NC_CACHE


def make_in_maps(qx, kx, vx, Wq, Wk, Wv):
    bf = ml_dtypes.bfloat16
    wqT = np.ascontiguousarray(Wq.astype(np.float32).T.astype(bf))
    wkT = np.ascontiguousarray(Wk.astype(np.float32).T.astype(bf))
    wvT = np.ascontiguousarray(Wv.astype(np.float32).T.astype(bf))
    in_maps = []
    for i in range(NC):
        rows = np.arange(RPC) * NC + i
        jp = np.arange(128)[:, None, None]
        cc = np.arange(8)[None, :, None]
        rl = np.arange(128)[None, None, :]
        mask = (8 * jp + cc <= 8 * rl + i).astype(bf)
        in_maps.append({
            "qxT": np.ascontiguousarray(qx[rows].T.astype(bf)),
            "kxT": np.ascontiguousarray(kx[rows].T.astype(bf)),
            "vxT": np.ascontiguousarray(vx[rows].T.astype(bf)),
            "wqT": wqT, "wkT": wkT, "wvT": wvT,
            "maskin": np.ascontiguousarray(mask),
        })
    return in_maps


def assemble(results):
    out = np.empty((N, D), np.float32)
    for i in range(NC):
        out[np.arange(RPC) * NC + i] = results[i]["y"]
    return out


def kernel(qx, kx, vx, Wq, Wk, Wv):
    nc = _get_nc()
    in_maps = make_in_maps(qx, kx, vx, Wq, Wk, Wv)
    res = run_bass_kernel_spmd(nc, in_maps, core_ids=list(range(NC)))
    return assemble(res.results)
